# revision 24
# baseline (speedup 1.0000x reference)
"""Trainium2 Bass kernel for EuclideanDistLoss.

reference:
    diff = latent1 - latent2                  # [B, D]
    d = sqrt(sum(diff^2, axis=1))             # [B]
    dev = d - CUTOFF
    penalty = where(dev > 0, dev^2, PRESSURE * dev^2)
    return mean(penalty)

Strategy: data-parallel over the batch dim across 8 NeuronCores. Each core
streams its 32768x256 shard of both inputs through SBUF ([128, k*256] tiles,
k rows per partition). Per tile: DVE subtract, then one ACT Square per
row-slice with accum_out writing the per-sample sum-of-squares directly
(fused square+reduce on ACT; DVE carries only the subtract, ~45% busy, so
compute hides under the DMA stream). A short tail computes the penalties and
a per-partition partial sum. The host sums the 8-core partials in float64 and
divides by the global batch (the "all-reduce" of the scalar).

The two input streams are issued on the two separate HWDGE rings (a-stream on
sync/qSPDynamicHW, b-stream on scalar/qActDynamicHW) with DMA issue emitted
pipeline_depth tiles ahead of compute so compute sem-waits never block issue.

Measured (hw-looped slope, median over rounds): 186-191 us/pass/core
depending on tunnel weather, vs a 183-184 us DMA-only floor measured in the
same sessions (~360 GB/s/core effective; two-stream read of 67.1 MiB/core,
8 cores concurrently = ~2.9 TB/s aggregate, i.e. at the HBM ceiling).
Probed and rejected: larger tiles (k=8/16/32), ring-alternating DMA issue,
bufs=16, SWDGE issue, mixed accum/DVE-reduce scheduling — none beat this.
"""

import numpy as np

B, D = 262144, 256
N_CORES = 8
P = 128
CUTOFF = 0.1
PRESSURE = 10.0

B_LOCAL = B // N_CORES  # 32768

# default per-tile schedule (rows per partition): uniform 512KB transfers.
# (A tapered end only shortens the single-pass tail, which is off the
# steady-state path, and costs extra small transfers per pass.)
K_DEFAULT = [4] * 64
BUFS_DEFAULT = 14
TAIL_UNITS = 8          # columns processed in the post-stream tail


def build_nc(b_local=B_LOCAL, k=K_DEFAULT, repeat=1, bufs=BUFS_DEFAULT,
             compute=True, b_engine="scalar", pipeline_depth=6,
             hw_loop=False, tail_units=TAIL_UNITS, accum=True, unroll=1,
             ring_alt=False):
    """Build + compile the per-core Bass program (SPMD: same program on all
    cores).

    repeat>1 re-runs the whole streaming pass over the same data (for
    benchmarking); hw_loop=True wraps the pass in a tc.For_i hardware loop so
    the program stays small at any repeat.
    compute=False builds a DMA-only variant (bandwidth ceiling probe).
    b_engine: which queue issues the latent2 stream ("sync" = same qSPDynamicHW
    ring as latent1, "scalar" = ACT's qActDynamicHW ring, "gpsimd" = SWDGE).
    pipeline_depth: tiles of DMA-issue lookahead emitted before compute of
    tile i (keeps compute sem-waits from blocking DMA issue on the same
    engine queue, which matters when b_engine="scalar" since ACT also runs
    Square).
    """
    import concourse.bacc as bacc
    import concourse.tile as tile
    from concourse import mybir

    f32 = mybir.dt.float32
    Alu = mybir.AluOpType
    Act = mybir.ActivationFunctionType

    if isinstance(k, int):
        tile_rows = P * k
        assert b_local % tile_rows == 0
        schedule = [k] * (b_local // tile_rows)
    else:  # explicit per-tile k schedule
        schedule = list(k)
        assert sum(schedule) * P == b_local
    T_units = sum(schedule)  # total k-units (= penalties per partition)
    n_tiles = len(schedule)

    # split point: columns [0, split) get their penalty math + partial-sum DMA
    # issued while the tapered end of the stream is still in flight; the
    # post-stream tail is a short chain over the last columns.
    split = max(T_units - tail_units, 0) if (compute and repeat == 1) else T_units
    n_out_cols = 2

    nc = bacc.Bacc("TRN2", target_bir_lowering=False, debug=False,
                   num_devices=N_CORES)
    a = nc.dram_tensor("latent1", [b_local, D], f32, kind="ExternalInput").ap()
    b = nc.dram_tensor("latent2", [b_local, D], f32, kind="ExternalInput").ap()
    out = nc.dram_tensor("out", [P, n_out_cols], f32, kind="ExternalOutput").ap()

    with tile.TileContext(nc) as tc:
        with (
            tc.tile_pool(name="pa", bufs=bufs) as pa,
            tc.tile_pool(name="pb", bufs=bufs) as pb,
            tc.tile_pool(name="keep", bufs=1) as keep,
        ):
            n = T_units  # penalties per partition
            ssq = keep.tile([P, n], f32)
            d_ = keep.tile([P, n], f32)
            mask = keep.tile([P, n], f32)  # 1.0 where d < CUTOFF
            fac = keep.tile([P, n], f32)   # 1 + (PRESSURE-1)*mask
            dd = keep.tile([P, n], f32)    # (d - CUTOFF)^2
            pen = keep.tile([P, n], f32)
            psum = keep.tile([P, n_out_cols], f32)
            neg_cut = keep.tile([P, 1], f32)
            nc.vector.memset(neg_cut, -CUTOFF)

            def penalty_ops(c_lo, c_hi, out_col):
                # critical path: Sqrt -> Square (both ACT, one table set) ->
                # mult -> reduce; mask/fac run on DVE in parallel with Square.
                s = slice(c_lo, c_hi)
                nc.scalar.activation(out=d_[:, s], in_=ssq[:, s], func=Act.Sqrt)
                nc.vector.tensor_scalar(mask[:, s], d_[:, s], CUTOFF, None,
                                        Alu.is_lt)
                nc.vector.tensor_scalar(
                    fac[:, s], mask[:, s], PRESSURE - 1.0, 1.0, Alu.mult, Alu.add
                )
                nc.scalar.activation(
                    out=dd[:, s], in_=d_[:, s], func=Act.Square, bias=neg_cut[:]
                )
                nc.vector.tensor_tensor(
                    out=pen[:, s], in0=dd[:, s], in1=fac[:, s], op=Alu.mult
                )
                nc.vector.tensor_reduce(
                    out=psum[:, out_col:out_col + 1], in_=pen[:, s],
                    axis=mybir.AxisListType.X, op=Alu.add,
                )
                nc.sync.dma_start(
                    out=out[:, out_col:out_col + 1],
                    in_=psum[:, out_col:out_col + 1],
                )

            if b_engine == "sync":
                b_eng = nc.sync
            elif b_engine == "scalar":
                b_eng = nc.scalar
            elif b_engine == "gpsimd":
                b_eng = nc.gpsimd
            else:
                raise ValueError(b_engine)

            # row offset / ssq column offset per tile index
            descs = []
            r0 = c0 = 0
            for kt in schedule:
                descs.append((r0, c0, kt))
                r0 += P * kt
                c0 += kt

            def issue_dma(i):
                r0, c0, kt = descs[i]
                a_v = a[r0:r0 + P * kt, :].rearrange("(p k) d -> p (k d)", p=P)
                b_v = b[r0:r0 + P * kt, :].rearrange("(p k) d -> p (k d)", p=P)
                ta = pa.tile([P, kt * D], f32, tag="ta")
                tb = pb.tile([P, kt * D], f32, tag="tb")
                if ring_alt:
                    # alternate each stream across both HWDGE rings per tile
                    ea = nc.sync if i % 2 == 0 else nc.scalar
                    eb = nc.scalar if i % 2 == 0 else nc.sync
                else:
                    ea, eb = nc.sync, b_eng
                ea.dma_start(out=ta, in_=a_v)
                eb.dma_start(out=tb, in_=b_v)
                return ta, tb

            emitted_bulk = [False]

            def compute_tile(i, ta, tb):
                _, c0, kt = descs[i]
                nc.vector.tensor_tensor(out=ta, in0=ta, in1=tb, op=Alu.subtract)
                use_accum = accum if accum != "mix" else (i % 2 == 0)
                if use_accum:
                    # ACT Square with fused per-partition sum: one ACTIVATE
                    # per row-slice writes ssq directly; DVE only subtracts.
                    for j in range(kt):
                        s = slice(j * D, (j + 1) * D)
                        nc.scalar.activation(
                            out=ta[:, s], in_=ta[:, s], func=Act.Square,
                            accum_out=ssq[:, c0 + j:c0 + j + 1],
                        )
                else:
                    nc.scalar.activation(out=ta, in_=ta, func=Act.Square)
                    nc.vector.tensor_reduce(
                        out=ssq[:, c0:c0 + kt],
                        in_=ta.rearrange("p (k d) -> p k d", d=D),
                        axis=mybir.AxisListType.X,
                        op=Alu.add,
                    )
                if (not emitted_bulk[0] and 0 < split < T_units
                        and c0 + kt >= split):
                    penalty_ops(0, split, 0)
                    emitted_bulk[0] = True

            def one_pass(with_penalty=False):
                # software-pipelined emission: DMA issue runs pipeline_depth
                # tiles ahead of compute so sem-waits on compute ops never
                # block DMA issue on the shared engine queues.
                depth = min(pipeline_depth, n_tiles) if compute else 0
                inflight = []
                for i in range(n_tiles):
                    inflight.append(issue_dma(i))
                    if not compute:
                        continue
                    if i >= depth:
                        compute_tile(i - depth, *inflight[i - depth])
                if compute:
                    for i in range(n_tiles - depth, n_tiles):
                        compute_tile(i, *inflight[i])
                if with_penalty:
                    # looped-bench mode: charge the full penalty chain to
                    # every pass so the slope measures a complete pass
                    penalty_ops(0, T_units, 0)

            if not compute:
                nc.vector.memset(psum, 0.0)
                nc.sync.dma_start(out=out, in_=psum)

            if hw_loop and repeat > 1:
                # unroll passes inside the loop body to amortize the
                # all-engine back-edge sync (pipeline drain) across them
                assert repeat % unroll == 0
                with tc.For_i(0, repeat // unroll, 1):
                    for _u in range(unroll):
                        one_pass(with_penalty=compute)
            else:
                for _r in range(repeat):
                    one_pass()
                if compute:
                    if split == T_units:
                        penalty_ops(0, T_units, 0)
                    else:
                        penalty_ops(split, T_units, 1)

    nc.compile()
    return nc


_NC_CACHE = {}


def _get_nc():
    key = "default"
    if key not in _NC_CACHE:
        _NC_CACHE[key] = build_nc()
    return _NC_CACHE[key]


def run_spmd(latent1, latent2, trace=False, **kwargs):
    """Shard inputs, run on 8 cores, return (scalar_loss, BassKernelResults)."""
    from concourse.bass_utils import run_bass_kernel_spmd

    nc = _get_nc()
    a = np.ascontiguousarray(np.asarray(latent1, dtype=np.float32))
    b = np.ascontiguousarray(np.asarray(latent2, dtype=np.float32))
    assert a.shape == (B, D) and b.shape == (B, D)
    in_maps = [
        {
            "latent1": a[c * B_LOCAL:(c + 1) * B_LOCAL],
            "latent2": b[c * B_LOCAL:(c + 1) * B_LOCAL],
        }
        for c in range(N_CORES)
    ]
    res = run_bass_kernel_spmd(
        nc, in_maps, core_ids=list(range(N_CORES)), trace=trace, **kwargs
    )
    total = sum(np.asarray(r["out"], dtype=np.float64).sum() for r in res.results)
    return np.asarray(total / B, dtype=np.float32), res


def kernel(latent1, latent2):
    loss, _ = run_spmd(latent1, latent2)
    return loss
